# revision 10
# baseline (speedup 1.0000x reference)
"""Trainium2 Bass kernel for nn_ChannelWiseMaxPoolWithCrossInfo.

Problem: x (8, 128, 64, 64) f32. 2x2 non-overlapping max-pool argmax per
channel c_pool gives, for each of the 1024 windows, the in-window position
(0..3) of the max. Output[b, c_pool, c_val, i] = the element of window i of
channel c_val at channel c_pool's argmax position. Shape (8, 128, 128, 1024).

Sharding: data-parallel over batch B=8 -> one batch element per NeuronCore.

Per-core algorithm (all shapes [partitions, free]):
  One-time setup:
  1. DMA x_b as X [128, 4096] f32.
  2. xstack [128, 32*128] bf16: the 4096 window rows r = 4*i + j, each row
     holding the 128 per-channel values of window i / position j, TRANSPOSED
     so row r lives at partition r%128, byte range [(r//128)*256, +256).
     Built with 32 PE transposes of strided X views + psum->sbuf bf16 casts.
  3. Exact f32 argmax (first-occurrence): e_j = (xw_j == max), index
     a = 3 - max(3*e0, 2*e1, e2), row index IDXf = 4*i + a   [128 c, 1024 i].
  4. Wrap IDXf into the SWDGE gather index layout W2[16g+p, s*128 + c] =
     IDX[c, 16s+p] (replicated over the 8 gpsimd index groups g): 8 PE
     transposes (IDXf -> [i, c] tiles) + 64 K=128 "fold" matmuls with 0/1
     selector matrices, cast f32->int16 on the psum->sbuf copy.
  Steady state (16 iterations x 8 pooled channels):
  5. DVE strided copy of 8 channels' wrapped index lists -> wc [128, 512] i16.
  6. One SBUF-source dma_gather (8192 indices x 256B rows, transpose mode):
     out tile [c_val=128, 8*1024] bf16 = the 8 output tiles.
  7. One SWDGE cast-DMA bf16->f32 straight to HBM out[c:c+8].
"""

import sys

sys.path.insert(0, "/opt/trn_rl_repo")

import numpy as np

import concourse.bacc as bacc
import concourse.mybir as mybir
import concourse.tile as tile
from concourse.alu_op_type import AluOpType
from concourse.bass_utils import run_bass_kernel_spmd

F32 = mybir.dt.float32
BF16 = mybir.dt.bfloat16
I16 = mybir.dt.int16

C = 128          # channels (both c_pool and c_val)
HW = 4096        # 64*64
P = 1024         # pooled positions (32*32)
GB = 8           # c_pools per gather batch
N_CORES = 8

_CACHE = {}


def _build_program(out_bf16: bool = False):
    nc = bacc.Bacc("TRN2", target_bir_lowering=False)

    x_d = nc.dram_tensor("x", [C, HW], F32, kind="ExternalInput")
    identf_d = nc.dram_tensor("identf", [C, C], F32, kind="ExternalInput")
    # fold[k, sp*128 + p] = 1.0 iff k == 16*sp + (p % 16): K=128 matmul with
    # lhsT = fold[:, sp*128:+128] replicates row block 16*sp+(p%16) of the rhs
    # into the wrapped-index partition layout.
    fold_d = nc.dram_tensor("fold", [C, 8 * C], F32, kind="ExternalInput")
    # iota4[p, i] = 4*i on every partition.
    iota4_d = nc.dram_tensor("iota4", [C, P], F32, kind="ExternalInput")
    out_dt = BF16 if out_bf16 else F32
    out_d = nc.dram_tensor("out", [C, C, P], out_dt, kind="ExternalOutput")

    with tile.TileContext(nc) as tc:
        with (
            tc.tile_pool(name="persist", bufs=1) as pp,
            tc.tile_pool(name="tmp", bufs=1) as tp,
            tc.tile_pool(name="gts", bufs=3) as gp,
            tc.tile_pool(name="wcs", bufs=3) as wp,
            tc.tile_pool(name="ps_t", bufs=2, space="PSUM") as psp,
            tc.tile_pool(name="ps_f", bufs=2, space="PSUM") as psf,
        ):
            X = pp.tile([C, HW], F32)
            nc.sync.dma_start(out=X[:], in_=x_d[:])
            identf = pp.tile([C, C], F32)
            nc.sync.dma_start(out=identf[:], in_=identf_d[:])
            foldm = pp.tile([C, 8 * C], F32)
            nc.sync.dma_start(out=foldm[:], in_=fold_d[:])
            iota4 = tp.tile([C, P], F32)
            nc.sync.dma_start(out=iota4[:], in_=iota4_d[:])

            # X viewed [c, hp(32), dh(2), wp(32), dw(2)]
            X5 = X.rearrange("c (hp dh wp dw) -> c hp dh wp dw",
                             hp=32, dh=2, wp=32, dw=2)

            # ---- xwS: window-interleaved x, col r = 4*i + j ----
            xwS = pp.tile([C, HW], F32)
            xwS5 = xwS.rearrange("c (hp wp dh dw) -> c hp wp dh dw",
                                 hp=32, wp=32, dh=2, dw=2)
            for j in range(4):
                nc.vector.tensor_copy(out=xwS5[:, :, :, j // 2, j % 2],
                                      in_=X5[:, :, j // 2, :, j % 2])

            # ---- xstack: transposed window rows (bf16) ----
            # xstack[p, rk*128 + v] = xwS[v, rk*128 + p]
            xstack = pp.tile([C, 32 * C], BF16)
            for rk in range(32):
                ps = psp.tile([C, C], F32, name="tps")
                nc.tensor.transpose(ps[:], xwS[:, rk * C:(rk + 1) * C],
                                    identf[:])
                nc.scalar.copy(xstack[:, rk * C:(rk + 1) * C], ps[:])

            # ---- exact f32 argmax -> row indices IDXf [c, i] ----
            xwS4 = xwS.rearrange("c (i four) -> c i four", four=4)
            xv = [xwS4[:, :, j] for j in range(4)]
            t0 = tp.tile([C, P], F32)
            t1 = tp.tile([C, P], F32)
            mx = tp.tile([C, P], F32)
            nc.vector.tensor_tensor(out=t0[:], in0=xv[0], in1=xv[1],
                                    op=AluOpType.max)
            nc.vector.tensor_tensor(out=t1[:], in0=xv[2], in1=xv[3],
                                    op=AluOpType.max)
            nc.vector.tensor_tensor(out=mx[:], in0=t0[:], in1=t1[:],
                                    op=AluOpType.max)
            e = []
            for j in range(3):
                ej = tp.tile([C, P], F32, name=f"e{j}")
                nc.vector.tensor_tensor(out=ej[:], in0=xv[j], in1=mx[:],
                                        op=AluOpType.is_equal)
                e.append(ej)
            # a = 3 - max(3*e0, 2*e1, e2)  (first-occurrence argmax)
            nc.vector.tensor_scalar_mul(out=t0[:], in0=e[0][:], scalar1=3.0)
            nc.vector.tensor_scalar_mul(out=t1[:], in0=e[1][:], scalar1=2.0)
            nc.vector.tensor_tensor(out=t0[:], in0=t0[:], in1=t1[:],
                                    op=AluOpType.max)
            nc.vector.tensor_tensor(out=t0[:], in0=t0[:], in1=e[2][:],
                                    op=AluOpType.max)
            nc.vector.tensor_scalar(out=t0[:], in0=t0[:], scalar1=-1.0,
                                    scalar2=3.0, op0=AluOpType.mult,
                                    op1=AluOpType.add)
            idxf = tp.tile([C, P], F32)
            nc.vector.tensor_tensor(out=idxf[:], in0=t0[:], in1=iota4[:],
                                    op=AluOpType.add)

            # ---- transpose IDXf into [i, c] tiles ----
            idxT = tp.tile([C, 8 * C], F32)
            for t in range(8):
                ps = psp.tile([C, C], F32, name="tps")
                nc.tensor.transpose(ps[:], idxf[:, t * C:(t + 1) * C],
                                    identf[:])
                nc.scalar.copy(idxT[:, t * C:(t + 1) * C], ps[:])

            # ---- fold into wrapped index layout W2 (int16) ----
            # W2[16g+p, (8t+sp)*128 + c] = IDXf[c, 128t + 16sp + p%16]
            W2 = pp.tile([C, 64 * C], I16)
            for t in range(8):
                for half in range(2):
                    pf = psf.tile([C, 4 * C], F32, name="fps")
                    for q in range(4):
                        sp = 4 * half + q
                        nc.tensor.matmul(pf[:, q * C:(q + 1) * C],
                                         foldm[:, sp * C:(sp + 1) * C],
                                         idxT[:, t * C:(t + 1) * C])
                    base = (8 * t + 4 * half) * C
                    nc.vector.tensor_copy(out=W2[:, base:base + 4 * C],
                                          in_=pf[:])

            # ---- steady state: gather + store, GB c_pools at a time ----
            W2r = W2.rearrange("p (s c) -> p c s", s=64, c=C)
            for g in range(C // GB):
                wc = wp.tile([C, GB * 64], I16, name="wc")
                wcv = wc.rearrange("p (k s) -> p k s", k=GB)
                nc.vector.tensor_copy(
                    out=wcv[:], in_=W2r[:, GB * g:GB * (g + 1), :])
                gt = gp.tile([C, GB * P], BF16, name="gt")
                gt3 = gt.rearrange("p (one n) -> p one n", one=1)
                nc.gpsimd.dma_gather(
                    out_ap=gt3[:],
                    in_ap=xstack[:],
                    idxs_ap=wc[:],
                    num_idxs=GB * P,
                    num_idxs_reg=GB * P,
                    elem_size=C,
                    transpose=True,
                    sbuf_tokens_per_rank=128,
                    sbuf_free_dim_per_rank=256,
                    single_packet=False,
                )
                ov = out_d.rearrange("k v i -> v k i")[:, GB * g:GB * (g + 1)]
                gtv = gt.rearrange("p (k i) -> p k i", k=GB)
                if out_bf16:
                    nc.sync.dma_start(out=ov, in_=gtv[:])
                else:
                    nc.gpsimd.dma_start(out=ov, in_=gtv[:])

    nc.compile()
    return nc


def get_program():
    if "nc" not in _CACHE:
        _CACHE["nc"] = _build_program()
    return _CACHE["nc"]


def make_aux_inputs() -> dict:
    identf = np.eye(C, dtype=np.float32)
    fold = np.zeros((C, 8 * C), dtype=np.float32)
    for sp in range(8):
        for p in range(C):
            fold[16 * sp + (p % 16), sp * C + p] = 1.0
    iota4 = np.broadcast_to(
        (4.0 * np.arange(P, dtype=np.float32))[None, :], (C, P)).copy()
    return {"identf": identf, "fold": fold, "iota4": iota4}


def kernel(x: np.ndarray) -> np.ndarray:
    assert x.shape == (N_CORES, C, 64, 64), x.shape
    x = np.ascontiguousarray(np.asarray(x, dtype=np.float32))
    nc = get_program()
    aux = make_aux_inputs()
    in_maps = [{"x": x[b].reshape(C, HW), **aux} for b in range(N_CORES)]
    res = run_bass_kernel_spmd(nc, in_maps, core_ids=list(range(N_CORES)))
    out = np.stack([np.asarray(res.results[b]["out"], dtype=np.float32)
                    for b in range(N_CORES)], axis=0)
    return out


# revision 11
# speedup vs baseline: 1.2037x; 1.2037x over previous
"""Plan C: PE mask-broadcast + DVE predicated select (bf16) + SWDGE cast store.

Per core (batch element):
  One-time: xwS f32 window-interleave; exact f32 masks e_j = (xw_j == max)
  as bf16; contiguous bf16 xw_j value tensors.
  Per c_pool: 6 K=1 matmuls broadcast mask rows -> PSUM f32; 4 ACT copies
  cast PSUM->SBUF bf16; DVE: base copy + 3 copy_predicated (bf16 2x mode);
  SWDGE cast-DMA bf16->f32 to HBM.
"""

import sys

sys.path.insert(0, "/opt/trn_rl_repo")

import numpy as np

import concourse.bacc as bacc
import concourse.mybir as mybir
import concourse.tile as tile
from concourse.alu_op_type import AluOpType
from concourse.bass_utils import run_bass_kernel_spmd

F32 = mybir.dt.float32
BF16 = mybir.dt.bfloat16
I32 = mybir.dt.int32

C = 128
HW = 4096
P = 1024
HALF = 512
N_CORES = 8

_CACHE = {}


def _build_program():
    nc = bacc.Bacc("TRN2", target_bir_lowering=False)

    x_d = nc.dram_tensor("x", [C, HW], F32, kind="ExternalInput")
    wsel_d = nc.dram_tensor("wsel", [C, C * C], BF16, kind="ExternalInput")
    out_d = nc.dram_tensor("out", [C, C, P], F32, kind="ExternalOutput")

    with tile.TileContext(nc) as tc:
        with (
            tc.tile_pool(name="persist", bufs=1) as pp,
            tc.tile_pool(name="tmp", bufs=1) as tp,
            tc.tile_pool(name="ots", bufs=4) as op,
            tc.tile_pool(name="mbs", bufs=3) as mp,
            tc.tile_pool(name="psum", bufs=2, space="PSUM") as psp,
        ):
            X = pp.tile([C, HW], F32)
            nc.sync.dma_start(out=X[:], in_=x_d[:])
            wsel = pp.tile([C, C * C], BF16)
            nc.sync.dma_start(out=wsel[:], in_=wsel_d[:])

            X5 = X.rearrange("c (hp dh wp dw) -> c hp dh wp dw",
                             hp=32, dh=2, wp=32, dw=2)

            # window-interleaved f32 copy (exact source for masks)
            xwS = pp.tile([C, HW], F32)
            xwS5 = xwS.rearrange("c (hp wp dh dw) -> c hp wp dh dw",
                                 hp=32, wp=32, dh=2, dw=2)
            for j in range(4):
                nc.vector.tensor_copy(out=xwS5[:, :, :, j // 2, j % 2],
                                      in_=X5[:, :, j // 2, :, j % 2])
            xwS4 = xwS.rearrange("c (i four) -> c i four", four=4)
            xv = [xwS4[:, :, j] for j in range(4)]

            # contiguous bf16 value tensors
            xwb = []
            for j in range(4):
                t = pp.tile([C, P], BF16, name=f"xwb{j}")
                nc.vector.tensor_copy(out=t[:], in_=xv[j])
                xwb.append(t)

            # exact f32 max -> bf16 equality masks
            t0 = tp.tile([C, P], F32)
            t1 = tp.tile([C, P], F32)
            mx = tp.tile([C, P], F32)
            nc.vector.tensor_tensor(out=t0[:], in0=xv[0], in1=xv[1],
                                    op=AluOpType.max)
            nc.vector.tensor_tensor(out=t1[:], in0=xv[2], in1=xv[3],
                                    op=AluOpType.max)
            nc.vector.tensor_tensor(out=mx[:], in0=t0[:], in1=t1[:],
                                    op=AluOpType.max)
            e = []
            for j in range(3):
                ej = pp.tile([C, P], BF16, name=f"e{j}")
                nc.vector.tensor_tensor(out=ej[:], in0=xv[j], in1=mx[:],
                                        op=AluOpType.is_equal)
                e.append(ej)

            for c in range(C):
                wc = wsel[:, c * C:(c + 1) * C]
                mb0 = mp.tile([C, P], BF16, name="mb0")
                mb1 = mp.tile([C, P], BF16, name="mb1")
                mb2 = mp.tile([C, P], BF16, name="mb2")
                mb = (mb0, mb1, mb2)
                for h in range(2):
                    sl = slice(h * HALF, (h + 1) * HALF)
                    ph = psp.tile([C, 3 * HALF], F32, name="ph")
                    for j in range(3):
                        nc.tensor.matmul(ph[:, j * HALF:(j + 1) * HALF],
                                         wc, e[j][:, sl])
                    for j in range(3):
                        nc.scalar.copy(mb[j][:, sl],
                                       ph[:, j * HALF:(j + 1) * HALF])

                if c % 4 == 0:
                    ot = op.tile([C, 4 * P], BF16, name="ot")
                osl = slice((c % 4) * P, (c % 4 + 1) * P)
                nc.vector.tensor_copy(out=ot[:, osl], in_=xwb[3][:])
                nc.vector.copy_predicated(out=ot[:, osl], mask=mb2.bitcast(mybir.dt.int16)[:],
                                          data=xwb[2][:])
                nc.vector.copy_predicated(out=ot[:, osl], mask=mb1.bitcast(mybir.dt.int16)[:],
                                          data=xwb[1][:])
                nc.vector.copy_predicated(out=ot[:, osl], mask=mb0.bitcast(mybir.dt.int16)[:],
                                          data=xwb[0][:])
                if c % 4 == 3:
                    ov = out_d.rearrange("k v i -> v k i")[:, c - 3:c + 1]
                    otv = ot.rearrange("p (k i) -> p k i", k=4)
                    nc.gpsimd.dma_start(out=ov, in_=otv[:])

    nc.compile()
    return nc


def get_program():
    if "nc" not in _CACHE:
        _CACHE["nc"] = _build_program()
    return _CACHE["nc"]


def make_aux_inputs() -> dict:
    import ml_dtypes
    w = np.zeros((C, C, C), dtype=ml_dtypes.bfloat16)
    for k in range(C):
        w[k, k, :] = 1.0
    return {"wsel": w.reshape(C, C * C)}


def kernel(x: np.ndarray) -> np.ndarray:
    assert x.shape == (N_CORES, C, 64, 64), x.shape
    x = np.ascontiguousarray(np.asarray(x, dtype=np.float32))
    nc = get_program()
    aux = make_aux_inputs()
    in_maps = [{"x": x[b].reshape(C, HW), **aux} for b in range(N_CORES)]
    res = run_bass_kernel_spmd(nc, in_maps, core_ids=list(range(N_CORES)))
    out = np.stack([np.asarray(res.results[b]["out"], dtype=np.float32)
                    for b in range(N_CORES)], axis=0)
    return out
